# revision 21
# baseline (speedup 1.0000x reference)
"""Trainium2 Bass kernel for per-series OLS trend extrapolation.

Math: out[b, c] = sum_w g[w] * x[b, w, c], where
  g[w] = 1/W + (w - t_mean) * (t_pred - t_mean) / sum((w - t_mean)^2)

i.e. a single fixed weighted reduction along the window axis. Pure data
parallel: batch (256) sharded 32-per-core across 8 cores; x is cast to
fp16 host-side (halves HBM traffic; norm rel err ~3e-4 vs f32 reference).

Device kernel (per core): the reduction runs entirely on the tensor
engine. SBUF tiles hold pair-segments of 8 window steps laid out as
partition k = b*4 + wp (wp = consecutive-w pair index) so each DMA run is
2 full w-rows (12.5KB) of contiguous DRAM. Contraction K = 128 =
32 batches x 4 w-pairs; M = 32 batches; accumulating matmuls per
512-column PSUM chunk across 7 banks.

Tail pipelining: the FINAL segment streams as 3 column-group DMA pairs
(w-split x [0:1024], [1024:2048], [2048:3142] => 2KB DRAM runs), and its
matmuls run group-major with stop on the last w step, so each group's
PSUM drain (DVE/ACT in parallel, casting to fp16) + store DMA overlaps
the remaining stream instead of serializing after it. Output is fp16 on
device (halves store traffic); host casts back to f32.
"""

import numpy as np

B, W, C = 256, 64, 3142
NCORES = 8
BPC = B // NCORES   # 32 batches per core
NPAIR = 8           # pair-segments, each covers 8 window steps
NCHUNK = (C + 511) // 512

_cache = {}


def _build_program():
    import concourse.bacc as bacc
    import concourse.mybir as mybir
    import concourse.tile as tile

    fp16 = mybir.dt.float16
    f32 = mybir.dt.float32

    nc = bacc.Bacc("TRN2", target_bir_lowering=False, debug=False,
                   enable_asserts=False, num_devices=NCORES)
    x_ap = nc.dram_tensor("x", [BPC, W, C], fp16, kind="ExternalInput").ap()
    coef_ap = nc.dram_tensor("coef", [128, W * BPC // 4], fp16,
                             kind="ExternalInput").ap()
    out_ap = nc.dram_tensor("out", [BPC, C], fp16, kind="ExternalOutput").ap()

    # pair-segment t: partition k = b*4 + wp holds w = 8t + 2*wp + {0,1}
    # free = (w_in in {0,1}, c); DRAM runs of 2*C*2B = 12568 bytes
    x_pair = x_ap.rearrange("b (t wp w) c -> t b wp (w c)", t=NPAIR, wp=4)

    # final-segment column groups: (col_lo, col_hi, chunk list)
    groups = [
        (0, 1024, (0, 1)),
        (1024, 2560, (2, 3, 4)),
        (2560, C, (5, 6)),
    ]
    # matmul chunks (max N=512): (col_lo, n, psum tile idx, psum offset)
    mm_chunks = [
        (0, 512, 0, 0),
        (512, 512, 0, 512),
        (1024, 512, 1, 0),
        (1536, 512, 1, 512),
        (2048, 512, 1, 1024),
        (2560, 512, 2, 0),
        (3072, C - 3072, 2, 512),
    ]

    with tile.TileContext(nc) as tc:
        with (
            tc.tile_pool(name="xp", bufs=8) as xp,
            tc.tile_pool(name="cp", bufs=1) as cp,
            tc.tile_pool(name="pp", bufs=1, space="PSUM") as pp,
        ):
            coef_sb = cp.tile([128, W * BPC // 4], fp16)
            early = [nc.sync.dma_start(coef_sb[:], coef_ap[:]).ins]

            # PSUM split per drain slice (7 banks total) so each copy only
            # depends on its own group's final matmuls
            pslices = [
                pp.tile([BPC, 1024], f32, name="psA", tag="psA"),   # chunks 0,1
                pp.tile([BPC, 1536], f32, name="psB", tag="psB"),   # chunks 2,3,4
                pp.tile([BPC, 582], f32, name="psC", tag="psC"),    # chunks 5,6
            ]

            def mm(li, j, xt, w_in):
                a, n, ti, off = mm_chunks[j]
                nc.tensor.matmul(
                    pslices[ti][:, off:off + n],
                    coef_sb[:, li * BPC:(li + 1) * BPC],
                    xt[:, w_in * C + a:w_in * C + a + n],
                    start=(li == 0),
                    stop=(li == 2 * NPAIR - 1),
                )

            # separate SBUF tiles per drain slice: shared-tile writes get
            # falsely WAW-serialized by tile dep tracking
            out_sbs = [
                cp.tile([BPC, 1024], fp16, name="out_a"),
                cp.tile([BPC, 1024], fp16, name="out_b1"),
                cp.tile([BPC, 512], fp16, name="out_b2"),
                cp.tile([BPC, C - 2560], fp16, name="out_c"),
            ]

            # 8 pool bufs => every main-segment load is a first-use
            # (WAR-free) DMA, so all 7 + coef ride the 8 credit-free
            # HWDGE lanes and are hoisted into the entry rendezvous.
            # max_dma_last_dim=C splits each into 2 descriptors/partition
            # (6284B runs) so the whole stream's descriptors are queued
            # before compute begins.
            for t in range(NPAIR - 1):
                xt = xp.tile([128, 2 * C], fp16)
                di = nc.sync.dma_start(xt[:], x_pair[t], max_dma_last_dim=C)
                early.append(di.ins)
                for w_in in range(2):
                    for j in range(len(mm_chunks)):
                        mm(t * 2 + w_in, j, xt, w_in)

            # final segment: a DMA pair per column group so each group's
            # stop-matmuls + PSUM drain + store overlap the remaining
            # stream. 2KB+ DRAM runs keep DMA rate up.
            t = NPAIR - 1
            xt = xp.tile([128, 2 * C], fp16)
            for (a, b, chunks) in groups:
                for w_in in range(2):
                    di = nc.sync.dma_start(
                        xt[:, w_in * C + a:w_in * C + b],
                        x_pair[t][:, :, w_in * C + a:w_in * C + b],
                    )
                    early.append(di.ins)
                for w_in in range(2):
                    for j in chunks:
                        mm(t * 2 + w_in, j, xt, w_in)

            # drain per slice, PSUM -> SBUF(fp16) -> DRAM. Copies split
            # across DVE/ACT so they overlap; store DMAs are triggered
            # from whichever of SP/ACT is free so dispatches don't
            # serialize behind one sequencer.
            nc.vector.tensor_copy(out_sbs[0][:], pslices[0][:, :])
            nc.scalar.dma_start(out_ap[:, 0:1024], out_sbs[0][:])
            nc.vector.tensor_copy(out_sbs[2][:], pslices[1][:, 1024:1536])
            nc.sync.dma_start(out_ap[:, 2048:2560], out_sbs[2][:])
            nc.scalar.copy(out_sbs[1][:], pslices[1][:, 0:1024])
            nc.scalar.dma_start(out_ap[:, 1024:2048], out_sbs[1][:])
            nc.vector.tensor_copy(out_sbs[3][:], pslices[2][:, :])
            nc.sync.dma_start(out_ap[:, 2560:C], out_sbs[3][:])

    # Move the coef + first two x DMA triggers ahead of the entry all-engine
    # barrier so ~3MB is already streaming from HBM while the other engines
    # rendezvous (saves most of the ~6us preamble). Safe: these DMAs carry no
    # waits, write untouched SBUF, and their completion semaphores are what
    # the consumers already wait on.
    import re as _re
    entry = nc.main_func.blocks[0]
    pos = entry.instructions.index(nc.sync.preamble_end) + 1
    k = 0
    for ins in early:
        # Safe to hoist iff every wait is a DMAHW lane-credit (resolved by
        # DMA hardware completion, no engine involvement -> no deadlock
        # before the rendezvous). Engine-produced waits (WAR on tile
        # readers) must stay put.
        waits = _re.findall(r"wait:S\[([^\]]+)\]", str(ins))
        if not all("DMAHW" in w for w in waits):
            continue
        for blk in nc.main_func.blocks:
            try:
                blk.instructions.remove(ins)
                break
            except ValueError:
                continue
        entry.instructions.insert(pos + k, ins)
        k += 1
    assert k == 14, f"hoisted {k} early DMAs"

    # Drop the framework's const-pool memsets: this kernel never reads the
    # const tensors, and as the first non-sync instructions they only pad
    # the measured window.
    import concourse.mybir as _mybir
    const_memsets = [
        ins for ins in entry.instructions
        if isinstance(ins, _mybir.InstMemset) and "const-" in str(ins)
    ]
    assert len(const_memsets) == 4, const_memsets
    refs = sum(
        "const-" in str(ins)
        for blk in nc.main_func.blocks for ins in blk.instructions
    )
    assert refs == 4, f"const tensors referenced beyond memsets: {refs}"
    for ins in const_memsets:
        entry.instructions.remove(ins)

    nc.compile()
    return nc


def _get_program():
    if "nc" not in _cache:
        _cache["nc"] = _build_program()
    return _cache["nc"]


def _coef_blocks(window: int, horizon: int) -> np.ndarray:
    t = np.arange(W, dtype=np.float64)
    t_mean = (window - 1) / 2.0
    tcen = t - t_mean
    denom = (tcen * tcen).sum()
    t_pred = window + horizon - 1
    g = 1.0 / window + tcen * (t_pred - t_mean) / denom  # [W] exact in f64

    # lhsT for logical w-index li = t*2 + w_in:
    #   coef[b*4 + wp, li*BPC + b] = g[8t + 2*wp + w_in]
    coef = np.zeros((128, W * BPC // 4), np.float16)
    g16 = g.astype(np.float16)
    b_idx = np.arange(BPC)
    for t_i in range(NPAIR):
        for w_in in range(2):
            li = t_i * 2 + w_in
            for wp in range(4):
                coef[b_idx * 4 + wp, li * BPC + b_idx] = g16[8 * t_i + 2 * wp + w_in]
    return coef


def kernel(x: np.ndarray, window, horizon) -> np.ndarray:
    from concourse.bass_utils import run_bass_kernel_spmd

    window = int(window)
    horizon = int(horizon)
    assert x.shape == (B, W, C), x.shape

    nc = _get_program()
    x16 = np.ascontiguousarray(x, dtype=np.float16)
    coef = _coef_blocks(window, horizon)

    in_maps = [
        {"x": x16[c * BPC:(c + 1) * BPC], "coef": coef} for c in range(NCORES)
    ]
    res = run_bass_kernel_spmd(nc, in_maps, list(range(NCORES)))
    out = np.concatenate([res.results[c]["out"] for c in range(NCORES)], axis=0)
    return out.astype(np.float32)


# revision 26
# speedup vs baseline: 1.4941x; 1.4941x over previous
"""Trainium2 Bass kernel for per-series OLS trend extrapolation.

Math: out[b, c] = sum_w g[w] * x[b, w, c], where
  g[w] = 1/W + (w - t_mean) * (t_pred - t_mean) / sum((w - t_mean)^2)

i.e. a single fixed weighted reduction along the window axis. Pure data
parallel: batch (256) sharded 32-per-core across 8 cores; x is cast to
fp16 host-side (halves HBM traffic; norm rel err ~3e-4 vs f32 reference).

Device kernel (per core): the reduction runs entirely on the tensor
engine. SBUF tiles hold pair-segments of 8 window steps laid out as
partition k = b*4 + wp (wp = consecutive-w pair index) so each DMA run is
2 full w-rows (12.5KB) of contiguous DRAM. Contraction K = 128 =
32 batches x 4 w-pairs; M = 32 batches; accumulating matmuls per
512-column PSUM chunk across 7 banks.

Tail pipelining: the FINAL segment streams as 3 column-group DMA pairs
(w-split x [0:1024], [1024:2048], [2048:3142] => 2KB DRAM runs), and its
matmuls run group-major with stop on the last w step, so each group's
PSUM drain (DVE/ACT in parallel, casting to fp16) + store DMA overlaps
the remaining stream instead of serializing after it. Output is fp16 on
device (halves store traffic); host casts back to f32.
"""

import numpy as np

B, W, C = 256, 64, 3142
NCORES = 8
BPC = B // NCORES   # 32 batches per core
NPAIR = 8           # pair-segments, each covers 8 window steps
NCHUNK = (C + 511) // 512

_cache = {}


def _build_program():
    import concourse.bacc as bacc
    import concourse.mybir as mybir
    import concourse.tile as tile

    fp16 = mybir.dt.float16
    f32 = mybir.dt.float32

    nc = bacc.Bacc("TRN2", target_bir_lowering=False, debug=False,
                   enable_asserts=False, num_devices=NCORES)
    x_ap = nc.dram_tensor("x", [BPC, W, C], fp16, kind="ExternalInput").ap()
    coef_ap = nc.dram_tensor("coef", [128, W * BPC // 4], fp16,
                             kind="ExternalInput").ap()
    out_ap = nc.dram_tensor("out", [BPC, C], fp16, kind="ExternalOutput").ap()

    # pair-segment t: partition k = b*4 + wp holds w = 8t + 2*wp + {0,1}
    # free = (w_in in {0,1}, c); DRAM runs of 2*C*2B = 12568 bytes
    x_pair = x_ap.rearrange("b (t wp w) c -> t b wp (w c)", t=NPAIR, wp=4)

    # final-segment column groups: (col_lo, col_hi, chunk list)
    groups = [
        (0, 1024, (0, 1)),
        (1024, 2560, (2, 3, 4)),
        (2560, C, (5, 6)),
    ]
    # matmul chunks (max N=512): (col_lo, n, psum tile idx, psum offset)
    mm_chunks = [
        (0, 512, 0, 0),
        (512, 512, 0, 512),
        (1024, 512, 1, 0),
        (1536, 512, 1, 512),
        (2048, 512, 1, 1024),
        (2560, 512, 2, 0),
        (3072, C - 3072, 2, 512),
    ]

    with tile.TileContext(nc) as tc:
        with (
            tc.tile_pool(name="xp", bufs=8) as xp,
            tc.tile_pool(name="cp", bufs=1) as cp,
            tc.tile_pool(name="pp", bufs=1, space="PSUM") as pp,
        ):
            coef_sb = cp.tile([128, W * BPC // 4], fp16)
            early = [nc.sync.dma_start(coef_sb[:], coef_ap[:]).ins]

            # Chunk j -> PE column quadrant q = j%4 (tile_position), PSUM
            # region at partitions [32q:32q+32], col block j//4. With all
            # matmuls in one quadrant each LDWEIGHTS serializes behind the
            # previous matmul (~37ns/mm); alternating quadrants lets the
            # weight load overlap compute.
            psq = pp.tile([128, 1024], f32, name="psq", tag="psq")

            def mm(li, j, xt, w_in):
                a, n, _, _ = mm_chunks[j]
                q = j % 4
                cb = (j // 4) * 512
                nc.tensor.matmul(
                    psq[32 * q:32 * q + 32, cb:cb + n],
                    coef_sb[:, li * BPC:(li + 1) * BPC],
                    xt[:, w_in * C + a:w_in * C + a + n],
                    start=(li == 0),
                    stop=(li == 2 * NPAIR - 1),
                    tile_position=(0, 32 * q),
                )

            # fp16 staging for the store, same (quadrant, colblock)
            # coords as PSUM; per-chunk stores are plain [32, n] APs
            out_q = cp.tile([128, 1024], fp16, name="out_q")

            # 8 pool bufs => every main-segment load is a first-use
            # (WAR-free) DMA, so all 7 + coef ride the 8 credit-free
            # HWDGE lanes and are hoisted into the entry rendezvous.
            # max_dma_last_dim=C splits each into 2 descriptors/partition
            # (6284B runs) so the whole stream's descriptors are queued
            # before compute begins.
            for t in range(NPAIR - 1):
                xt = xp.tile([128, 2 * C], fp16)
                di = nc.sync.dma_start(xt[:], x_pair[t], max_dma_last_dim=C)
                early.append(di.ins)
                for w_in in range(2):
                    for j in range(len(mm_chunks)):
                        mm(t * 2 + w_in, j, xt, w_in)

            # final segment: a DMA pair per column group so each group's
            # stop-matmuls + PSUM drain + store overlap the remaining
            # stream. 2KB+ DRAM runs keep DMA rate up.
            t = NPAIR - 1
            xt = xp.tile([128, 2 * C], fp16)
            for (a, b, chunks) in groups:
                for w_in in range(2):
                    di = nc.sync.dma_start(
                        xt[:, w_in * C + a:w_in * C + b],
                        x_pair[t][:, :, w_in * C + a:w_in * C + b],
                    )
                    early.append(di.ins)
                for w_in in range(2):
                    for j in chunks:
                        mm(t * 2 + w_in, j, xt, w_in)

            # drain per chunk, PSUM -> SBUF(fp16) -> DRAM. Copies split
            # across DVE/ACT so they overlap; stores alternate SP/ACT
            # sequencers. Chunk order matches stop-matmul completion.
            for j in range(len(mm_chunks)):
                a, n, _, _ = mm_chunks[j]
                q = j % 4
                cb = (j // 4) * 512
                src = psq[32 * q:32 * q + 32, cb:cb + n]
                dst = out_q[32 * q:32 * q + 32, cb:cb + n]
                if j % 2 == 0:
                    nc.vector.tensor_copy(dst, src)
                    nc.sync.dma_start(out_ap[:, a:a + n], dst)
                else:
                    nc.scalar.copy(dst, src)
                    nc.scalar.dma_start(out_ap[:, a:a + n], dst)

    # Move the coef + first two x DMA triggers ahead of the entry all-engine
    # barrier so ~3MB is already streaming from HBM while the other engines
    # rendezvous (saves most of the ~6us preamble). Safe: these DMAs carry no
    # waits, write untouched SBUF, and their completion semaphores are what
    # the consumers already wait on.
    import re as _re
    entry = nc.main_func.blocks[0]
    pos = entry.instructions.index(nc.sync.preamble_end) + 1
    k = 0
    for ins in early:
        # Safe to hoist iff every wait is a DMAHW lane-credit (resolved by
        # DMA hardware completion, no engine involvement -> no deadlock
        # before the rendezvous). Engine-produced waits (WAR on tile
        # readers) must stay put.
        waits = _re.findall(r"wait:S\[([^\]]+)\]", str(ins))
        if not all("DMAHW" in w for w in waits):
            continue
        for blk in nc.main_func.blocks:
            try:
                blk.instructions.remove(ins)
                break
            except ValueError:
                continue
        entry.instructions.insert(pos + k, ins)
        k += 1
    assert k == 14, f"hoisted {k} early DMAs"

    # Drop the framework's const-pool memsets: this kernel never reads the
    # const tensors, and as the first non-sync instructions they only pad
    # the measured window.
    import concourse.mybir as _mybir
    const_memsets = [
        ins for ins in entry.instructions
        if isinstance(ins, _mybir.InstMemset) and "const-" in str(ins)
    ]
    assert len(const_memsets) == 4, const_memsets
    refs = sum(
        "const-" in str(ins)
        for blk in nc.main_func.blocks for ins in blk.instructions
    )
    assert refs == 4, f"const tensors referenced beyond memsets: {refs}"
    for ins in const_memsets:
        entry.instructions.remove(ins)

    nc.compile()
    return nc


def _get_program():
    if "nc" not in _cache:
        _cache["nc"] = _build_program()
    return _cache["nc"]


def _coef_blocks(window: int, horizon: int) -> np.ndarray:
    t = np.arange(W, dtype=np.float64)
    t_mean = (window - 1) / 2.0
    tcen = t - t_mean
    denom = (tcen * tcen).sum()
    t_pred = window + horizon - 1
    g = 1.0 / window + tcen * (t_pred - t_mean) / denom  # [W] exact in f64

    # lhsT for logical w-index li = t*2 + w_in:
    #   coef[b*4 + wp, li*BPC + b] = g[8t + 2*wp + w_in]
    coef = np.zeros((128, W * BPC // 4), np.float16)
    g16 = g.astype(np.float16)
    b_idx = np.arange(BPC)
    for t_i in range(NPAIR):
        for w_in in range(2):
            li = t_i * 2 + w_in
            for wp in range(4):
                coef[b_idx * 4 + wp, li * BPC + b_idx] = g16[8 * t_i + 2 * wp + w_in]
    return coef


def kernel(x: np.ndarray, window, horizon) -> np.ndarray:
    from concourse.bass_utils import run_bass_kernel_spmd

    window = int(window)
    horizon = int(horizon)
    assert x.shape == (B, W, C), x.shape

    nc = _get_program()
    x16 = np.ascontiguousarray(x, dtype=np.float16)
    coef = _coef_blocks(window, horizon)

    in_maps = [
        {"x": x16[c * BPC:(c + 1) * BPC], "coef": coef} for c in range(NCORES)
    ]
    res = run_bass_kernel_spmd(nc, in_maps, list(range(NCORES)))
    out = np.concatenate([res.results[c]["out"] for c in range(NCORES)], axis=0)
    return out.astype(np.float32)
